# revision 1
# baseline (speedup 1.0000x reference)
"""Trainium2 Bass kernel for a GPT-style transformer block (B=2,T=2048,C=768,H=12).

Sharding: 8 cores; core c handles batch b=c//4, query block qo=(c%4)*512.
Each core gets its batch's x feature-major [C,T], rolled so its 512 query
tokens are columns 0:512.  K/V are computed for all 2048 keys (duplicated
across the 4 cores of a batch -- no cross-core communication); Q/attention/
MLP only for the 512 queries.

Numerics: bf16 for the attention path (x->q/k/v, scores, AV -- softmax
averaging makes these errors negligible), fp32r (TF32) matmuls for the MLP,
exact fp32 residual path.

LayerNorm folding: LN scale/bias fold into the following weights on the
host.  On device projections run on raw x:
    proj(xn) = istd[t] * ( (W^T x)[d,t] + (-mu[t]) * colsum(W)[d] )
so normalization costs one rank-1 PE accumulation plus a fused multiply at
PSUM copyback -- xn is never materialized.  Softmax uses an augmented-V
column of ones so the AV matmul also produces sum(exp); reciprocals are
exp(-ln(x)) on the scalar engine.
"""
import sys

sys.path.insert(0, "/opt/trn_rl_repo")

import numpy as np
import ml_dtypes

import concourse.bass as bass
import concourse.tile as tile
from concourse import bacc, mybir
from concourse.bass_utils import run_bass_kernel_spmd

F32 = mybir.dt.float32
F32R = mybir.dt.float32r
BF16 = mybir.dt.bfloat16
AF = mybir.ActivationFunctionType
ALU = mybir.AluOpType

B, T, C, H = 2, 2048, 768, 12
HD = C // H             # 64
C4 = 4 * C              # 3072
EPS = 1e-5
NCORES = 8
TQ = (B * T) // NCORES  # 512
PC = C // 128           # 6
PC4 = C4 // 128         # 24
NT4 = T // 512          # 4
NSC = T // 128          # 16
NBIAS = (5 * C + C4) // 128  # 54


def _round_tf32(a):
    b = np.ascontiguousarray(a, dtype=np.float32).view(np.uint32)
    b = (b + 0x1000 + ((b >> 13) & 1)) & np.uint32(0xFFFFE000)
    return b.view(np.float32)


def _build(has_qkv_bias, has_o_bias, has_proj_bias, has_fc_bias, has_mask, reps=1):
    has_bias_any = has_qkv_bias or has_o_bias or has_proj_bias or has_fc_bias
    nc = bacc.Bacc()

    x_d = nc.dram_tensor("x_fm", [C, T], F32, kind="ExternalInput")
    xb_d = nc.dram_tensor("x_bf", [C, T], BF16, kind="ExternalInput")
    wq_d = nc.dram_tensor("wq", [C, C], BF16, kind="ExternalInput")
    wk_d = nc.dram_tensor("wk", [C, C], BF16, kind="ExternalInput")
    wv_d = nc.dram_tensor("wv", [C, C], BF16, kind="ExternalInput")
    wo_d = nc.dram_tensor("wo", [HD, H, C], BF16, kind="ExternalInput")
    wfc_d = nc.dram_tensor("wfc", [PC4, 128, PC, 128], F32R, kind="ExternalInput")
    wproj_d = nc.dram_tensor("wproj", [C4, C], F32R, kind="ExternalInput")
    wsum3_d = nc.dram_tensor("wsums3", [1, 3 * C], BF16, kind="ExternalInput")
    wsumfc_d = nc.dram_tensor("wsumsfc", [1, C4], F32R, kind="ExternalInput")
    bias_d = nc.dram_tensor("biases", [128, NBIAS], F32, kind="ExternalInput")
    brow_d = nc.dram_tensor("bias_rows", [1, 3 * C], BF16, kind="ExternalInput")
    mask_d = nc.dram_tensor("maskb", [128, NSC], F32, kind="ExternalInput")
    out_d = nc.dram_tensor("out_fm", [C, TQ], F32, kind="ExternalOutput")

    x_pot = x_d.rearrange("(o p) t -> p o t", p=128)
    xb_pot = xb_d.rearrange("(o p) t -> p o t", p=128)

    with tile.TileContext(nc) as tc:
      for _rep in range(reps):
        with tc.tile_pool(name=f"const{_rep}", bufs=1) as const, \
             tc.tile_pool(name=f"persist{_rep}", bufs=1) as persist:

            # ---------------- constants ----------------
            ones_f = const.tile([128, 1], F32)
            nc.vector.memset(ones_f[:], 1.0)
            ones_col_b = const.tile([128, 1], BF16)
            nc.vector.memset(ones_col_b[:], 1.0)
            ones_col_r = const.tile([128, 1], F32R)
            nc.vector.tensor_copy(ones_col_r[:], ones_f[:])
            onesr_f = const.tile([1, 128], F32)
            nc.vector.memset(onesr_f[:], 1.0)
            ones_row = const.tile([1, 128], F32R)
            nc.vector.tensor_copy(ones_row[:], onesr_f[:])
            ones_row_b = const.tile([1, 128], BF16)
            nc.vector.memset(ones_row_b[:], 1.0)

            if has_bias_any:
                bias_sb = const.tile([128, NBIAS], F32)
                nc.sync.dma_start(bias_sb[:], bias_d[:, :])
            if has_mask:
                mask_sb = const.tile([128, NSC], F32)
                nc.sync.dma_start(mask_sb[:], mask_d[:, :])
            if has_qkv_bias:
                brow_sb = const.tile([1, 3 * C], BF16)
                nc.sync.dma_start(brow_sb[:], brow_d[:, :])

            x_q = persist.tile([128, PC, TQ], F32)   # exact residual copy
            nc.sync.dma_start(x_q[:], x_pot[:, :, 0:TQ])

            with tc.tile_pool(name=f"attp{_rep}", bufs=1) as attp:
                q_bf = attp.tile([128, PC, TQ], BF16)
                k_bf = attp.tile([128, PC, T], BF16)
                vt_aug = attp.tile([128, NSC, H * (HD + 1)], BF16)

                # ======== phases A+B: LN1 stats + QKV projections ========
                with tc.tile_pool(name=f"phB{_rep}", bufs=1) as phB, \
                     tc.tile_pool(name=f"rtmp{_rep}", bufs=2) as rtmp, \
                     tc.tile_pool(name=f"sqp{_rep}", bufs=3) as sqp, \
                     tc.tile_pool(name=f"wcyc{_rep}", bufs=2) as wcyc:
                  with tc.tile_pool(name=f"st_ps{_rep}", bufs=2, space="PSUM") as st_ps:

                      x_bf = phB.tile([128, PC, T], BF16)
                      nc.sync.dma_start(x_bf[:], xb_pot[:, :, :])
                      istd_b = phB.tile([128, T], BF16)
                      istd_col = phB.tile([128, NSC], F32)
                      risd_r = phB.tile([1, T], BF16)  # 1/istd (bias path only)

                      for t4 in range(NT4):
                          sl = slice(t4 * 512, (t4 + 1) * 512)
                          p1 = st_ps.tile([1, 512], F32, tag="p1")
                          for j in range(PC):
                              nc.tensor.matmul(p1[:], ones_col_b[:], x_bf[:, j, sl],
                                               start=(j == 0), stop=(j == PC - 1))
                          mean_c = rtmp.tile([1, 512], F32, tag="rt")
                          nc.vector.tensor_scalar_mul(mean_c[:], p1[:], 1.0 / C)
                          negmu_c = rtmp.tile([1, 512], BF16, tag="rtb")
                          nc.vector.tensor_scalar_mul(negmu_c[:], mean_c[:], -1.0)
                          # center x in place: x_bf -= mean (broadcast via PE)
                          nm_ps = st_ps.tile([128, 512], F32, tag="nm")
                          nc.tensor.matmul(nm_ps[:], ones_row_b[:], negmu_c[:],
                                           start=True, stop=True)
                          for j in range(PC):
                              nc.vector.tensor_tensor(x_bf[:, j, sl], x_bf[:, j, sl],
                                                      nm_ps[:], ALU.add)
                          # var = E[(x-mu)^2]
                          p2 = st_ps.tile([1, 512], F32, tag="p2")
                          for j in range(PC):
                              xsq = sqp.tile([128, 512], BF16, tag="xsq")
                              nc.vector.tensor_mul(xsq[:], x_bf[:, j, sl], x_bf[:, j, sl])
                              nc.tensor.matmul(p2[:], ones_col_b[:], xsq[:],
                                               start=(j == 0), stop=(j == PC - 1))
                          var_c = rtmp.tile([1, 512], F32, tag="rt")
                          nc.vector.tensor_scalar(var_c[:], p2[:], 1.0 / C, EPS,
                                                  ALU.mult, ALU.add)
                          lnv_c = rtmp.tile([1, 512], F32, tag="rt")
                          nc.scalar.activation(lnv_c[:], var_c[:], AF.Ln)
                          istd_c = rtmp.tile([1, 512], F32, tag="rt")
                          nc.scalar.activation(istd_c[:], lnv_c[:], AF.Exp, scale=-0.5)
                          istd_cb = rtmp.tile([1, 512], BF16, tag="rtb")
                          nc.vector.tensor_copy(istd_cb[:], istd_c[:])
                          if has_qkv_bias:
                              nc.scalar.activation(risd_r[:, sl], lnv_c[:], AF.Exp, scale=0.5)
                          # broadcast istd across partitions
                          bp = st_ps.tile([128, 512], F32, tag="bp")
                          nc.tensor.matmul(bp[:], ones_row_b[:], istd_cb[:],
                                           start=True, stop=True)
                          nc.vector.tensor_copy(istd_b[:, sl], bp[:])
                          # istd token-major [128, 4 cols] for the V copyback
                          for o in range(4):
                              nc.sync.dma_start(istd_col[:, t4 * 4 + o:t4 * 4 + o + 1],
                                                istd_c[0:1, o * 128:(o + 1) * 128])

                  with tc.tile_pool(name=f"qkv_ps{_rep}", bufs=2, space="PSUM") as qkv_ps:
                        # ---- V (all keys, token-major, augmented ones col) ----
                        wv_sb = wcyc.tile([128, PC, C], BF16, tag="w")
                        nc.sync.dma_start(wv_sb[:], wv_d.rearrange("(o p) m -> p o m", p=128))
                        for sc in range(NSC):
                            ssl = slice(sc * 128, (sc + 1) * 128)
                            nc.vector.memset(
                                vt_aug[:, sc, :].rearrange("p (h e) -> p h e", e=HD + 1)[:, :, HD:HD + 1],
                                1.0)
                            for half in range(2):
                                hsl = slice(half * 384, (half + 1) * 384)
                                vp = qkv_ps.tile([128, 512], F32, tag="pp", name="vp")[:, 0:384]
                                for j in range(PC):
                                    nc.tensor.matmul(vp[:], x_bf[:, j, ssl], wv_sb[:, j, hsl],
                                                     start=(j == 0),
                                                     stop=(j == PC - 1 and not has_qkv_bias))
                                if has_qkv_bias:
                                    nc.tensor.matmul(
                                        vp[:], risd_r[:, ssl],
                                        brow_sb[:, 2 * C + half * 384:2 * C + (half + 1) * 384],
                                        start=False, stop=True)
                                dst = vt_aug[:, sc, :].rearrange("p (h e) -> p h e", e=HD + 1)[
                                    :, half * 6:(half + 1) * 6, 0:HD]
                                nc.vector.tensor_scalar(
                                    dst, vp[:].rearrange("p (h e) -> p h e", e=HD),
                                    istd_col[:, sc:sc + 1], None, ALU.mult)

                        # ---- Q (queries only) ----
                        wq_sb = wcyc.tile([128, PC, C], BF16, tag="w")
                        nc.sync.dma_start(wq_sb[:], wq_d.rearrange("(o p) m -> p o m", p=128))
                        for oc in range(PC):
                            qp = qkv_ps.tile([128, 512], F32, tag="pp", name="qp")
                            for j in range(PC):
                                nc.tensor.matmul(qp[:], wq_sb[:, j, oc * 128:(oc + 1) * 128],
                                                 x_bf[:, j, 0:TQ], start=(j == 0),
                                                 stop=(j == PC - 1 and not has_qkv_bias))
                            if has_qkv_bias:
                                nc.tensor.matmul(qp[:], brow_sb[:, oc * 128:(oc + 1) * 128],
                                                 risd_r[:, 0:TQ], start=False, stop=True)
                            nc.vector.tensor_tensor(q_bf[:, oc, :], qp[:], istd_b[:, 0:TQ],
                                                    ALU.mult)
                        # ---- K (all keys; oc-outer so k chunks finish early) ----
                        wk_sb = wcyc.tile([128, PC, C], BF16, tag="w")
                        nc.sync.dma_start(wk_sb[:], wk_d.rearrange("(o p) m -> p o m", p=128))
                        for oc in range(PC):
                            for t4 in range(NT4):
                                sl = slice(t4 * 512, (t4 + 1) * 512)
                                kp = qkv_ps.tile([128, 512], F32, tag="pp", name="kp")
                                for j in range(PC):
                                    nc.tensor.matmul(kp[:], wk_sb[:, j, oc * 128:(oc + 1) * 128],
                                                     x_bf[:, j, sl], start=(j == 0),
                                                     stop=(j == PC - 1 and not has_qkv_bias))
                                if has_qkv_bias:
                                    nc.tensor.matmul(kp[:],
                                                     brow_sb[:, C + oc * 128:C + (oc + 1) * 128],
                                                     risd_r[:, sl], start=False, stop=True)
                                nc.vector.tensor_tensor(k_bf[:, oc, sl], kp[:], istd_b[:, sl],
                                                        ALU.mult)
                # ============ phase C: attention ============
                wo_sb = persist.tile([HD, H, C], BF16)
                nc.sync.dma_start(wo_sb[:], wo_d[:, :, :])

                with tc.tile_pool(name=f"ypool{_rep}", bufs=1) as ypool:
                    y_sb = ypool.tile([HD + 1, H, TQ], BF16)
                    y_nm = ypool.tile([HD, H, TQ], BF16)

                    with tc.tile_pool(name=f"sc_ps{_rep}", bufs=2, space="PSUM") as sc_ps, \
                         tc.tile_pool(name=f"y_psp{_rep}", bufs=2, space="PSUM") as y_psp, \
                         tc.tile_pool(name=f"attb{_rep}", bufs=3) as attb:
                        for h in range(H):
                            base = 64 * (h & 1)
                            ch = h // 2
                            yp = y_psp.tile([HD + 1, TQ], F32, tag="yp")
                            for scp in range(NSC // 2):
                                sp = sc_ps.tile([128, 2, 512], F32, tag="sp")
                                for i in range(2):
                                    sc = 2 * scp + i
                                    nc.tensor.matmul(
                                        sp[:, i, :],
                                        k_bf[base:base + HD, ch, sc * 128:(sc + 1) * 128],
                                        q_bf[base:base + HD, ch, :],
                                        start=True, stop=True)
                                att = attb.tile([128, 2, 512], BF16, tag="att")
                                if has_mask:
                                    for i in range(2):
                                        sc = 2 * scp + i
                                        nc.scalar.activation(att[:, i, :], sp[:, i, :], AF.Exp,
                                                             bias=mask_sb[:, sc:sc + 1])
                                else:
                                    nc.scalar.activation(att[:], sp[:], AF.Exp)
                                for i in range(2):
                                    sc = 2 * scp + i
                                    nc.tensor.matmul(yp[:], vt_aug[:, sc, 65 * h:65 * h + 65],
                                                     att[:, i, :],
                                                     start=(sc == 0), stop=(sc == NSC - 1))
                            nc.vector.tensor_copy(y_sb[:, h, :], yp[:])

                    # softmax denominators: recip = exp(-ln(sumexp))
                    with tc.tile_pool(name=f"crow{_rep}", bufs=1) as crow, \
                         tc.tile_pool(name=f"nr_ps{_rep}", bufs=2, space="PSUM") as nr_ps:
                        serow_b = crow.tile([1, H * TQ], BF16)
                        nc.sync.dma_start(serow_b[:], y_sb[HD:HD + 1, :, :])
                        lnrow = crow.tile([1, H * TQ], F32)
                        nc.scalar.activation(lnrow[:], serow_b[:], AF.Ln)
                        recips_b = crow.tile([1, H * TQ], BF16)
                        nc.scalar.activation(recips_b[:], lnrow[:], AF.Exp, scale=-1.0)
                        for h in range(H):
                            rp = nr_ps.tile([HD, TQ], F32, tag="rp")
                            nc.tensor.matmul(rp[:], ones_row_b[:, 0:HD],
                                             recips_b[:, h * TQ:(h + 1) * TQ],
                                             start=True, stop=True)
                            nc.vector.tensor_tensor(y_nm[:, h, :], y_sb[0:HD, h, :], rp[:],
                                                    ALU.mult)

                    # output projection + residual -> x2
                    x2 = persist.tile([128, PC, TQ], F32)
                    x2_r = persist.tile([128, PC, TQ], F32R)
                    with tc.tile_pool(name=f"wo_ps{_rep}", bufs=3, space="PSUM") as wo_ps:
                        for oc in range(PC):
                            op = wo_ps.tile([128, TQ], F32, tag="op")
                            for h in range(H):
                                nc.tensor.matmul(op[:], wo_sb[:, h, oc * 128:(oc + 1) * 128],
                                                 y_nm[:, h, :], start=(h == 0), stop=(h == H - 1))
                            if has_o_bias:
                                nc.scalar.activation(op[:], op[:], AF.Identity,
                                                     bias=bias_sb[:, 3 * PC + oc:3 * PC + oc + 1])
                            nc.vector.tensor_tensor(x2[:, oc, :], op[:], x_q[:, oc, :], ALU.add)
                            nc.vector.tensor_copy(x2_r[:, oc, :], x2[:, oc, :])

            # ============ phase D: LN2 + MLP ============
            out_sb = persist.tile([128, PC, TQ], F32)
            with tc.tile_pool(name=f"drow{_rep}", bufs=1) as drow, \
                 tc.tile_pool(name=f"dtmp{_rep}", bufs=2) as dtmp:
              with tc.tile_pool(name=f"d_ps{_rep}", bufs=1, space="PSUM") as d_ps:
                  wsumfc = drow.tile([1, C4], F32R)
                  nc.sync.dma_start(wsumfc[:], wsumfc_d[:, :])
                  p1 = d_ps.tile([1, TQ], F32, tag="p1")
                  p2 = d_ps.tile([1, TQ], F32, tag="p2")
                  for j in range(PC):
                      nc.tensor.matmul(p1[:], ones_col_r[:], x2_r[:, j, :],
                                       start=(j == 0), stop=(j == PC - 1))
                  for j in range(PC):
                      xsq2 = dtmp.tile([128, TQ], F32R, tag="xsq2")
                      nc.vector.tensor_mul(xsq2[:], x2_r[:, j, :], x2_r[:, j, :])
                      nc.tensor.matmul(p2[:], ones_col_r[:], xsq2[:],
                                       start=(j == 0), stop=(j == PC - 1))
                  mean2 = dtmp.tile([1, TQ], F32, tag="dt")
                  nc.vector.tensor_scalar_mul(mean2[:], p1[:], 1.0 / C)
                  msq2 = dtmp.tile([1, TQ], F32, tag="dt")
                  nc.vector.tensor_tensor(msq2[:], mean2[:], mean2[:], ALU.mult)
                  var2 = dtmp.tile([1, TQ], F32, tag="dt")
                  nc.vector.tensor_scalar(var2[:], p2[:], 1.0 / C, EPS, ALU.mult, ALU.add)
                  nc.vector.tensor_sub(var2[:], var2[:], msq2[:])
                  lnv2 = dtmp.tile([1, TQ], F32, tag="dt")
                  nc.scalar.activation(lnv2[:], var2[:], AF.Ln)
                  istd2 = dtmp.tile([1, TQ], F32, tag="dt")
                  nc.scalar.activation(istd2[:], lnv2[:], AF.Exp, scale=-0.5)
                  negmu2_r = drow.tile([1, TQ], F32R)
                  nc.vector.tensor_scalar_mul(negmu2_r[:], mean2[:], -1.0)
                  istd2_r = drow.tile([1, TQ], F32R)
                  nc.vector.tensor_copy(istd2_r[:], istd2[:])
                  bp2 = d_ps.tile([128, TQ], F32, tag="bp2")
                  nc.tensor.matmul(bp2[:], ones_row[:], istd2_r[:], start=True, stop=True)
                  istd2_b = drow.tile([128, TQ], F32)
                  nc.vector.tensor_copy(istd2_b[:], bp2[:])

              with tc.tile_pool(name=f"pr_ps{_rep}", bufs=1, space="PSUM") as pr_ps, \
                   tc.tile_pool(name=f"fc_ps{_rep}", bufs=2, space="PSUM") as fc_ps, \
                   tc.tile_pool(name=f"h_sb{_rep}", bufs=3) as h_sb, \
                   tc.tile_pool(name=f"w_sb2{_rep}", bufs=4) as w_sb2:
                    prs = [pr_ps.tile([128, TQ], F32, tag=f"pr{i}", name=f"pr{i}")
                           for i in range(PC)]
                    for kc in range(PC4):
                        wfcc = w_sb2.tile([128, PC, 128], F32R, tag="wfcc")
                        nc.sync.dma_start(wfcc[:], wfc_d[kc])
                        fp = fc_ps.tile([128, TQ], F32, tag="fp")
                        for j in range(PC):
                            nc.tensor.matmul(fp[:], wfcc[:, j, :], x2_r[:, j, :],
                                             start=(j == 0), stop=False)
                        nc.tensor.matmul(fp[:], wsumfc[:, kc * 128:(kc + 1) * 128],
                                         negmu2_r[:], start=False, stop=True)
                        nc.vector.tensor_tensor(fp[:], fp[:], istd2_b[:], ALU.mult)
                        hc = h_sb.tile([128, TQ], F32R, tag="hc")
                        if has_fc_bias:
                            nc.scalar.activation(hc[:], fp[:], AF.Gelu,
                                                 bias=bias_sb[:, 5 * PC + kc:5 * PC + kc + 1])
                        else:
                            nc.scalar.activation(hc[:], fp[:], AF.Gelu)
                        wpc = w_sb2.tile([128, C], F32R, tag="wpc")
                        nc.sync.dma_start(wpc[:], wproj_d[kc * 128:(kc + 1) * 128, :])
                        for oc in range(PC):
                            nc.tensor.matmul(prs[oc][:], wpc[:, oc * 128:(oc + 1) * 128],
                                             hc[:], start=(kc == 0), stop=(kc == PC4 - 1))
                    for oc in range(PC):
                        if has_proj_bias:
                            nc.scalar.activation(prs[oc][:], prs[oc][:], AF.Identity,
                                                 bias=bias_sb[:, 4 * PC + oc:4 * PC + oc + 1])
                        nc.vector.tensor_tensor(out_sb[:, oc, :], prs[oc][:], x2[:, oc, :],
                                                ALU.add)
            nc.sync.dma_start(out_d.rearrange("(o p) t -> p o t", p=128), out_sb[:])

    nc.compile()
    return nc


_CACHE = {}


def _get_program(flags, reps=1):
    key = (flags, reps)
    if key not in _CACHE:
        _CACHE[key] = _build(*flags, reps=reps)
    return _CACHE[key]


def kernel(**inputs) -> np.ndarray:
    x = np.asarray(inputs["x"], dtype=np.float32)
    padding_mask = np.asarray(inputs["padding_mask"])
    ln1_s = np.asarray(inputs["ln1_scale"], dtype=np.float32)
    ln1_b = np.asarray(inputs["ln1_bias"], dtype=np.float32)
    ln2_s = np.asarray(inputs["ln2_scale"], dtype=np.float32)
    ln2_b = np.asarray(inputs["ln2_bias"], dtype=np.float32)
    Wq = np.asarray(inputs["Wq"], dtype=np.float32)
    Wk = np.asarray(inputs["Wk"], dtype=np.float32)
    Wv = np.asarray(inputs["Wv"], dtype=np.float32)
    bq = np.asarray(inputs["bq"], dtype=np.float32)
    bk = np.asarray(inputs["bk"], dtype=np.float32)
    bv = np.asarray(inputs["bv"], dtype=np.float32)
    Wo = np.asarray(inputs["Wo"], dtype=np.float32)
    bo = np.asarray(inputs["bo"], dtype=np.float32)
    Wfc = np.asarray(inputs["Wfc"], dtype=np.float32)
    bfc = np.asarray(inputs["bfc"], dtype=np.float32)
    Wproj = np.asarray(inputs["Wproj"], dtype=np.float32)
    bproj = np.asarray(inputs["bproj"], dtype=np.float32)

    sc_q = 1.0 / np.sqrt(HD)
    Wq_f = Wq.transpose(1, 0, 2).reshape(C, C)
    Wk_f = Wk.transpose(1, 0, 2).reshape(C, C)
    Wv_f = Wv.transpose(1, 0, 2).reshape(C, C)
    wq_eff = (ln1_s[:, None] * Wq_f * sc_q).astype(ml_dtypes.bfloat16)
    wk_eff = (ln1_s[:, None] * Wk_f).astype(ml_dtypes.bfloat16)
    wv_eff = (ln1_s[:, None] * Wv_f).astype(ml_dtypes.bfloat16)
    bq_eff = (ln1_b @ Wq_f) * sc_q + bq.reshape(C) * sc_q
    bk_eff = ln1_b @ Wk_f + bk.reshape(C)
    bv_eff = ln1_b @ Wv_f + bv.reshape(C)
    wfc_eff = _round_tf32(ln2_s[:, None] * Wfc)
    bfc_eff = ln2_b @ Wfc + bfc
    wfc_pre = np.ascontiguousarray(
        wfc_eff.reshape(PC, 128, PC4, 128).transpose(2, 1, 0, 3))
    wproj_r = _round_tf32(Wproj)
    wo_pre = np.ascontiguousarray(
        Wo.reshape(H, HD, C).transpose(1, 0, 2)).astype(ml_dtypes.bfloat16)

    # column sums of the weights as the PE sees them (bf16 for q/k/v)
    wsums3 = np.concatenate(
        [wq_eff.astype(np.float32).sum(0), wk_eff.astype(np.float32).sum(0),
         wv_eff.astype(np.float32).sum(0)]).astype(ml_dtypes.bfloat16)[None, :]
    wsumsfc = _round_tf32(wfc_eff.sum(0))[None, :]

    biases = np.concatenate([bq_eff, bk_eff, bv_eff, bo, bproj, bfc_eff])
    bias_pre = np.ascontiguousarray(biases.reshape(NBIAS, 128).T).astype(np.float32)
    brows = np.concatenate([bq_eff, bk_eff, bv_eff]).astype(ml_dtypes.bfloat16)[None, :]

    has_qkv_bias = bool(np.abs(np.concatenate([bq_eff, bk_eff, bv_eff])).max() > 0)
    has_o_bias = bool(np.abs(bo).max() > 0)
    has_proj_bias = bool(np.abs(bproj).max() > 0)
    has_fc_bias = bool(np.abs(bfc_eff).max() > 0)
    has_mask = bool(padding_mask.any())

    nc = _get_program((has_qkv_bias, has_o_bias, has_proj_bias, has_fc_bias, has_mask))

    shared = {
        "wq": wq_eff, "wk": wk_eff, "wv": wv_eff, "wo": wo_pre,
        "wfc": wfc_pre, "wproj": wproj_r, "wsums3": wsums3, "wsumsfc": wsumsfc, "biases": bias_pre,
        "bias_rows": brows,
    }
    in_maps = []
    for c in range(NCORES):
        b, qo = c // (NCORES // B), (c % (NCORES // B)) * TQ
        xr = np.roll(x[b], -qo, axis=0)
        x_fm = np.ascontiguousarray(xr.T)
        x_bf = x_fm.astype(ml_dtypes.bfloat16)
        mrow = np.roll(padding_mask[b], -qo)
        maskb = np.ascontiguousarray(
            np.where(mrow, -1e30, 0.0).astype(np.float32).reshape(NSC, 128).T)
        in_maps.append({**shared, "x_fm": x_fm, "x_bf": x_bf, "maskb": maskb})

    res = run_bass_kernel_spmd(nc, in_maps, core_ids=list(range(NCORES)))

    out = np.empty((B, T, C), dtype=np.float32)
    for c in range(NCORES):
        b, qo = c // (NCORES // B), (c % (NCORES // B)) * TQ
        out[b, qo:qo + TQ, :] = res.results[c]["out_fm"].T
    return out



# revision 16
# speedup vs baseline: 1.0266x; 1.0266x over previous
"""Trainium2 Bass kernel for a GPT-style transformer block (B=2,T=2048,C=768,H=12).

Sharding: 8 cores; core c handles batch b=c//4, query block qo=(c%4)*512.
Each core gets its batch's x feature-major [C,T], rolled so its 512 query
tokens are columns 0:512.  K/V are computed for all 2048 keys (duplicated
across the 4 cores of a batch -- no cross-core communication); Q/attention/
MLP only for the 512 queries.

Numerics: bf16 attention path and bf16 MLP weights (softmax averaging and
the 2e-2 budget make these errors negligible), exact fp32 residual path.

Engine balance: LN stats use PE colsums + Act Rsqrt; elementwise work is
split across DVE (nc.vector), Pool (nc.gpsimd) and Act (squares / scaled
copies) so the PE stream stays back-to-back (p-state!).  Softmax uses an
augmented-V ones column so AV also produces sum(exp); reciprocals are DVE
InstReciprocal (no Act table switches).
"""
import sys

sys.path.insert(0, "/opt/trn_rl_repo")

import numpy as np
import ml_dtypes

import concourse.bass as bass
import concourse.tile as tile
from concourse import bacc, mybir
from concourse.bass_utils import run_bass_kernel_spmd

F32 = mybir.dt.float32
F32R = mybir.dt.float32r
BF16 = mybir.dt.bfloat16
AF = mybir.ActivationFunctionType
ALU = mybir.AluOpType

B, T, C, H = 2, 2048, 768, 12
HD = C // H             # 64
C4 = 4 * C              # 3072
EPS = 1e-5
NCORES = 8
TQ = (B * T) // NCORES  # 512
PC = C // 128           # 6
PC4 = C4 // 128         # 24
NT4 = T // 512          # 4
NSC = T // 128          # 16
NBIAS = (5 * C + C4) // 128  # 54


def _build(has_qkv_bias, has_o_bias, has_proj_bias, has_fc_bias, has_mask, reps=1):
    has_bias_any = has_qkv_bias or has_o_bias or has_proj_bias or has_fc_bias
    nc = bacc.Bacc()

    x_d = nc.dram_tensor("x_fm", [C, T], F32, kind="ExternalInput")
    xb_d = nc.dram_tensor("x_bf", [C, T], BF16, kind="ExternalInput")
    wq_d = nc.dram_tensor("wq", [C, C], BF16, kind="ExternalInput")
    wk_d = nc.dram_tensor("wk", [C, C], BF16, kind="ExternalInput")
    wv_d = nc.dram_tensor("wv", [C, C], BF16, kind="ExternalInput")
    wo_d = nc.dram_tensor("wo", [HD, H, C], BF16, kind="ExternalInput")
    wfc_d = nc.dram_tensor("wfc", [PC4, 128, PC, 128], BF16, kind="ExternalInput")
    wproj_d = nc.dram_tensor("wproj", [C4, C], BF16, kind="ExternalInput")
    bias_d = nc.dram_tensor("biases", [128, NBIAS], F32, kind="ExternalInput")
    brow_d = nc.dram_tensor("bias_rows", [1, 3 * C], BF16, kind="ExternalInput")
    mask_d = nc.dram_tensor("maskb", [128, NSC], F32, kind="ExternalInput")
    out_d = nc.dram_tensor("out_fm", [C, TQ], F32, kind="ExternalOutput")

    x_pot = x_d.rearrange("(o p) t -> p o t", p=128)
    xb_pot = xb_d.rearrange("(o p) t -> p o t", p=128)

    with tile.TileContext(nc) as tc:
      for _rep in range(reps):
        with tc.tile_pool(name=f"const{_rep}", bufs=1) as const, \
             tc.tile_pool(name=f"persist{_rep}", bufs=1) as persist:

            # ---------------- constants ----------------
            ones_f = const.tile([128, 1], F32)
            nc.vector.memset(ones_f[:], 1.0)
            ones_col_b = const.tile([128, 1], BF16)
            nc.vector.memset(ones_col_b[:], 1.0)
            ones_col_r = const.tile([128, 1], F32R)
            nc.vector.tensor_copy(ones_col_r[:], ones_f[:])
            onesr_f = const.tile([1, 128], F32)
            nc.vector.memset(onesr_f[:], 1.0)
            ones_row = const.tile([1, 128], F32R)
            nc.vector.tensor_copy(ones_row[:], onesr_f[:])
            ones_row_b = const.tile([1, 128], BF16)
            nc.vector.memset(ones_row_b[:], 1.0)

            if has_bias_any:
                bias_sb = const.tile([128, NBIAS], F32)
                nc.sync.dma_start(bias_sb[:], bias_d[:, :])
            if has_mask:
                mask_sb = const.tile([128, NSC], F32)
                nc.sync.dma_start(mask_sb[:], mask_d[:, :])
            if has_qkv_bias:
                brow_sb = const.tile([1, 3 * C], BF16)
                nc.sync.dma_start(brow_sb[:], brow_d[:, :])

            x_q = persist.tile([128, PC, TQ], F32)   # exact residual copy
            nc.sync.dma_start(x_q[:], x_pot[:, :, 0:TQ])

            with tc.tile_pool(name=f"attp{_rep}", bufs=1) as attp:
                q_bf = attp.tile([128, PC, TQ], BF16)
                k_bf = attp.tile([128, PC, T], BF16)
                vt_aug = attp.tile([128, NSC, H * (HD + 1)], BF16)

                # ======== phases A+B: LN1 stats + QKV projections ========
                with tc.tile_pool(name=f"phB{_rep}", bufs=1) as phB, \
                     tc.tile_pool(name=f"rtmp{_rep}", bufs=3) as rtmp, \
                     tc.tile_pool(name=f"sqp{_rep}", bufs=4) as sqp, \
                     tc.tile_pool(name=f"wcyc{_rep}", bufs=2) as wcyc:
                  with tc.tile_pool(name=f"st_ps{_rep}", bufs=2, space="PSUM") as st_ps:

                      x_bf = phB.tile([128, PC, T], BF16)
                      nc.sync.dma_start(x_bf[:], xb_pot[:, :, :])
                      istd_b = phB.tile([128, T], BF16)
                      istd_col = phB.tile([128, NSC], F32)
                      risd_r = phB.tile([1, T], BF16)  # 1/istd (bias path only)

                      for t4 in range(NT4):
                          sl = slice(t4 * 512, (t4 + 1) * 512)
                          p1 = st_ps.tile([1, 512], F32, tag="p1")
                          for j in range(PC):
                              nc.tensor.matmul(p1[:], ones_col_b[:], x_bf[:, j, sl],
                                               start=(j == 0), stop=(j == PC - 1))
                          negmu_c = rtmp.tile([1, 512], BF16, tag="rtb")
                          nc.vector.tensor_scalar_mul(negmu_c[:], p1[:], -1.0 / C)
                          # center x in place: x_bf -= mean (broadcast via PE)
                          nm_ps = st_ps.tile([128, 512], F32, tag="nm")
                          nc.tensor.matmul(nm_ps[:], ones_row_b[:], negmu_c[:],
                                           start=True, stop=True)
                          nm_sb = sqp.tile([128, 512], BF16, tag="nmsb")
                          nc.vector.tensor_copy(nm_sb[:], nm_ps[:])
                          for j in range(PC):
                              eng = nc.gpsimd if j >= 3 else nc.vector
                              eng.tensor_tensor(x_bf[:, j, sl], x_bf[:, j, sl],
                                                nm_sb[:], ALU.add)
                          # var = E[(x-mu)^2]; squares on the Act engine
                          p2 = st_ps.tile([1, 512], F32, tag="p2")
                          for j in range(PC):
                              xsq = sqp.tile([128, 512], BF16, tag="xsq")
                              nc.scalar.activation(xsq[:], x_bf[:, j, sl], AF.Square)
                              nc.tensor.matmul(p2[:], ones_col_b[:], xsq[:],
                                               start=(j == 0), stop=(j == PC - 1))
                          var_c = rtmp.tile([1, 512], F32, tag="rt")
                          nc.vector.tensor_scalar(var_c[:], p2[:], 1.0 / C, EPS,
                                                  ALU.mult, ALU.add)
                          sd_c = rtmp.tile([1, 512], F32, tag="rt")
                          nc.scalar.activation(sd_c[:], var_c[:], AF.Sqrt)
                          istd_c = rtmp.tile([1, 512], F32, tag="rt")
                          nc.vector.reciprocal(istd_c[:], sd_c[:])
                          istd_cb = rtmp.tile([1, 512], BF16, tag="rtb")
                          nc.vector.tensor_copy(istd_cb[:], istd_c[:])
                          if has_qkv_bias:
                              nc.scalar.activation(risd_r[:, sl], var_c[:], AF.Sqrt)
                          # broadcast istd across partitions
                          bp = st_ps.tile([128, 512], F32, tag="bp")
                          nc.tensor.matmul(bp[:], ones_row_b[:], istd_cb[:],
                                           start=True, stop=True)
                          nc.vector.tensor_copy(istd_b[:, sl], bp[:])
                          # istd token-major [128, 4 cols] for the V copyback
                          for o in range(4):
                              nc.sync.dma_start(istd_col[:, t4 * 4 + o:t4 * 4 + o + 1],
                                                istd_c[0:1, o * 128:(o + 1) * 128])

                  with tc.tile_pool(name=f"qkv_ps{_rep}", bufs=2, space="PSUM") as qkv_ps:
                        # ---- V (all keys, token-major, augmented ones col) ----
                        wv_sb = wcyc.tile([128, PC, C], BF16, tag="w")
                        nc.sync.dma_start(wv_sb[:], wv_d.rearrange("(o p) m -> p o m", p=128))
                        for sc in range(NSC):
                            ssl = slice(sc * 128, (sc + 1) * 128)
                            nc.vector.memset(
                                vt_aug[:, sc, :].rearrange("p (h e) -> p h e", e=HD + 1)[:, :, HD:HD + 1],
                                1.0)
                            for half in range(2):
                                hsl = slice(half * 384, (half + 1) * 384)
                                vp = qkv_ps.tile([128, 512], F32, tag="pp", name="vp")[:, 0:384]
                                for j in range(PC):
                                    nc.tensor.matmul(vp[:], x_bf[:, j, ssl], wv_sb[:, j, hsl],
                                                     start=(j == 0),
                                                     stop=(j == PC - 1 and not has_qkv_bias))
                                if has_qkv_bias:
                                    nc.tensor.matmul(
                                        vp[:], risd_r[:, ssl],
                                        brow_sb[:, 2 * C + half * 384:2 * C + (half + 1) * 384],
                                        start=False, stop=True)
                                dst = vt_aug[:, sc, :].rearrange("p (h e) -> p h e", e=HD + 1)[
                                    :, half * 6:(half + 1) * 6, 0:HD]
                                if half == 0:
                                    nc.scalar.activation(
                                        dst, vp[:].rearrange("p (h e) -> p h e", e=HD),
                                        AF.Copy, scale=istd_col[:, sc:sc + 1])
                                else:
                                    nc.vector.tensor_scalar(
                                        dst, vp[:].rearrange("p (h e) -> p h e", e=HD),
                                        istd_col[:, sc:sc + 1], None, ALU.mult)

                        # ---- Q (queries only) ----
                        wq_sb = wcyc.tile([128, PC, C], BF16, tag="w")
                        nc.sync.dma_start(wq_sb[:], wq_d.rearrange("(o p) m -> p o m", p=128))
                        for oc in range(PC):
                            qp = qkv_ps.tile([128, 512], F32, tag="pp", name="qp")
                            for j in range(PC):
                                nc.tensor.matmul(qp[:], wq_sb[:, j, oc * 128:(oc + 1) * 128],
                                                 x_bf[:, j, 0:TQ], start=(j == 0),
                                                 stop=(j == PC - 1 and not has_qkv_bias))
                            if has_qkv_bias:
                                nc.tensor.matmul(qp[:], brow_sb[:, oc * 128:(oc + 1) * 128],
                                                 risd_r[:, 0:TQ], start=False, stop=True)
                            nc.vector.tensor_tensor(q_bf[:, oc, :], qp[:], istd_b[:, 0:TQ],
                                                    ALU.mult)
                        # ---- K (all keys; oc-outer so k chunks finish early) ----
                        wk_sb = wcyc.tile([128, PC, C], BF16, tag="w")
                        nc.sync.dma_start(wk_sb[:], wk_d.rearrange("(o p) m -> p o m", p=128))
                        for oc in range(PC):
                            for t4 in range(NT4):
                                sl = slice(t4 * 512, (t4 + 1) * 512)
                                kp = qkv_ps.tile([128, 512], F32, tag="pp", name="kp")
                                for j in range(PC):
                                    nc.tensor.matmul(kp[:], wk_sb[:, j, oc * 128:(oc + 1) * 128],
                                                     x_bf[:, j, sl], start=(j == 0),
                                                     stop=(j == PC - 1 and not has_qkv_bias))
                                if has_qkv_bias:
                                    nc.tensor.matmul(kp[:],
                                                     brow_sb[:, C + oc * 128:C + (oc + 1) * 128],
                                                     risd_r[:, sl], start=False, stop=True)
                                nc.vector.tensor_tensor(k_bf[:, oc, sl], kp[:], istd_b[:, sl],
                                                        ALU.mult)
                # ============ phase C: attention ============
                wo_sb = persist.tile([HD, H, C], BF16)
                nc.sync.dma_start(wo_sb[:], wo_d[:, :, :])

                with tc.tile_pool(name=f"ypool{_rep}", bufs=1) as ypool:
                    y_sb = ypool.tile([HD, H, TQ], BF16)
                    y_nm = ypool.tile([HD, H, TQ], BF16)
                    recf = ypool.tile([1, H * TQ], F32R)

                    with tc.tile_pool(name=f"sc_ps{_rep}", bufs=2, space="PSUM") as sc_ps, \
                         tc.tile_pool(name=f"y_psp{_rep}", bufs=2, space="PSUM") as y_psp, \
                         tc.tile_pool(name=f"attb{_rep}", bufs=3) as attb:
                        for h in range(H):
                            base = 64 * (h & 1)
                            ch = h // 2
                            yp = y_psp.tile([HD + 1, TQ], F32, tag="yp")
                            for scp in range(NSC // 2):
                                sp = sc_ps.tile([128, 2, 512], F32, tag="sp")
                                for i in range(2):
                                    sc = 2 * scp + i
                                    nc.tensor.matmul(
                                        sp[:, i, :],
                                        k_bf[base:base + HD, ch, sc * 128:(sc + 1) * 128],
                                        q_bf[base:base + HD, ch, :],
                                        start=True, stop=True)
                                att = attb.tile([128, 2, 512], BF16, tag="att")
                                if has_mask:
                                    for i in range(2):
                                        sc = 2 * scp + i
                                        nc.scalar.activation(att[:, i, :], sp[:, i, :], AF.Exp,
                                                             bias=mask_sb[:, sc:sc + 1])
                                else:
                                    nc.scalar.activation(att[:], sp[:], AF.Exp)
                                for i in range(2):
                                    sc = 2 * scp + i
                                    nc.tensor.matmul(yp[:], vt_aug[:, sc, 65 * h:65 * h + 65],
                                                     att[:, i, :],
                                                     start=(sc == 0), stop=(sc == NSC - 1))
                            # denominators: reciprocal on DVE (no Act tables)
                            with nc.allow_low_precision("softmax recip in tf32"):
                                nc.vector.reciprocal(
                                    recf[:, h * TQ:(h + 1) * TQ], yp[HD:HD + 1, :])
                            if h % 2:
                                nc.scalar.activation(y_sb[:, h, :], yp[0:HD, :], AF.Copy)
                            else:
                                nc.vector.tensor_copy(y_sb[:, h, :], yp[0:HD, :])

                    # Wo + per-head normalization, interleaved so PE stays hot
                    x2 = persist.tile([128, PC, TQ], F32R)
                    with tc.tile_pool(name=f"wo_ps{_rep}", bufs=1, space="PSUM") as wo_ps, \
                         tc.tile_pool(name=f"rp_ps{_rep}", bufs=2, space="PSUM") as rp_ps:
                        wops = [wo_ps.tile([128, TQ], F32, tag=f"op{i}", name=f"op{i}")
                                for i in range(PC)]
                        rps = {}
                        for h in range(H):
                            rp = rp_ps.tile([HD, TQ], F32, tag="rp")
                            nc.tensor.matmul(rp[:], ones_row[:, 0:HD],
                                             recf[:, h * TQ:(h + 1) * TQ],
                                             start=True, stop=True)
                            rps[h] = rp
                            nc.vector.tensor_tensor(y_nm[:, h, :], y_sb[:, h, :], rp[:],
                                                    ALU.mult)
                            if h >= 1:
                                hh = h - 1
                                for oc in range(PC):
                                    nc.tensor.matmul(wops[oc][:],
                                                     wo_sb[:, hh, oc * 128:(oc + 1) * 128],
                                                     y_nm[:, hh, :], start=(hh == 0),
                                                     stop=False)
                        for oc in range(PC):
                            nc.tensor.matmul(wops[oc][:],
                                             wo_sb[:, H - 1, oc * 128:(oc + 1) * 128],
                                             y_nm[:, H - 1, :], start=False, stop=True)
                        for oc in range(PC):
                            if has_o_bias:
                                nc.scalar.activation(wops[oc][:], wops[oc][:], AF.Identity,
                                                     bias=bias_sb[:, 3 * PC + oc:3 * PC + oc + 1])
                            nc.vector.tensor_tensor(x2[:, oc, :], wops[oc][:], x_q[:, oc, :],
                                                    ALU.add)

            # ============ phase D: LN2 + MLP ============
            out_sb = persist.tile([128, PC, TQ], F32)
            xc_bf = persist.tile([128, PC, TQ], BF16)
            with tc.tile_pool(name=f"drow{_rep}", bufs=1) as drow, \
                 tc.tile_pool(name=f"dtmp{_rep}", bufs=3) as dtmp:
              with tc.tile_pool(name=f"d_ps{_rep}", bufs=1, space="PSUM") as d_ps:
                  p1 = d_ps.tile([1, TQ], F32, tag="p1")
                  p2 = d_ps.tile([1, TQ], F32, tag="p2")
                  for j in range(PC):
                      nc.tensor.matmul(p1[:], ones_col_r[:], x2[:, j, :],
                                       start=(j == 0), stop=(j == PC - 1))
                  for j in range(PC):
                      xsq2 = dtmp.tile([128, TQ], F32R, tag="xsq2")
                      nc.scalar.activation(xsq2[:], x2[:, j, :], AF.Square)
                      nc.tensor.matmul(p2[:], ones_col_r[:], xsq2[:],
                                       start=(j == 0), stop=(j == PC - 1))
                  mean2 = dtmp.tile([1, TQ], F32, tag="dt")
                  nc.vector.tensor_scalar_mul(mean2[:], p1[:], 1.0 / C)
                  msq2 = dtmp.tile([1, TQ], F32, tag="dt")
                  nc.vector.tensor_tensor(msq2[:], mean2[:], mean2[:], ALU.mult)
                  var2 = dtmp.tile([1, TQ], F32, tag="dt")
                  nc.vector.tensor_scalar(var2[:], p2[:], 1.0 / C, EPS, ALU.mult, ALU.add)
                  nc.vector.tensor_sub(var2[:], var2[:], msq2[:])
                  sd2 = dtmp.tile([1, TQ], F32, tag="dt")
                  nc.scalar.activation(sd2[:], var2[:], AF.Sqrt)
                  istd2 = dtmp.tile([1, TQ], F32, tag="dt")
                  nc.vector.reciprocal(istd2[:], sd2[:])
                  negmu2_r = drow.tile([1, TQ], F32R)
                  nc.vector.tensor_scalar_mul(negmu2_r[:], mean2[:], -1.0)
                  istd2_r = drow.tile([1, TQ], F32R)
                  nc.vector.tensor_copy(istd2_r[:], istd2[:])
                  bp2 = d_ps.tile([128, TQ], F32, tag="bp2")
                  nc.tensor.matmul(bp2[:], ones_row[:], istd2_r[:], start=True, stop=True)
                  istd2_b = drow.tile([128, TQ], F32)
                  nc.vector.tensor_copy(istd2_b[:], bp2[:])
                  nm2_ps = d_ps.tile([128, TQ], F32, tag="nm2")
                  nc.tensor.matmul(nm2_ps[:], ones_row[:], negmu2_r[:], start=True, stop=True)
                  nm2_b = drow.tile([128, TQ], F32)
                  nc.vector.tensor_copy(nm2_b[:], nm2_ps[:])
                  # center + scale + cast: xc = (x2 - mu) * istd  (bf16)
                  for j in range(PC):
                      xct = dtmp.tile([128, TQ], F32, tag="xct")
                      eng, eng2 = (nc.vector, nc.gpsimd) if j % 2 == 0 else (nc.gpsimd, nc.vector)
                      eng.tensor_tensor(xct[:], x2[:, j, :], nm2_b[:], ALU.add)
                      eng2.tensor_tensor(xc_bf[:, j, :], xct[:], istd2_b[:], ALU.mult)

              with tc.tile_pool(name=f"pr_ps{_rep}", bufs=1, space="PSUM") as pr_ps, \
                   tc.tile_pool(name=f"fc_ps{_rep}", bufs=2, space="PSUM") as fc_ps, \
                   tc.tile_pool(name=f"h_sb{_rep}", bufs=3) as h_sb, \
                   tc.tile_pool(name=f"w_sb2{_rep}", bufs=4) as w_sb2:
                    prs = [pr_ps.tile([128, TQ], F32, tag=f"pr{i}", name=f"pr{i}")
                           for i in range(PC)]
                    hcs = {}
                    for kc in range(PC4):
                        wfcc = w_sb2.tile([128, PC, 128], BF16, tag="wfcc")
                        nc.sync.dma_start(wfcc[:], wfc_d[kc])
                        fp = fc_ps.tile([128, TQ], F32, tag="fp")
                        for j in range(PC):
                            nc.tensor.matmul(fp[:], wfcc[:, j, :], xc_bf[:, j, :],
                                             start=(j == 0), stop=(j == PC - 1))
                        hc = h_sb.tile([128, TQ], BF16, tag="hc")
                        if has_fc_bias:
                            nc.scalar.activation(hc[:], fp[:], AF.Gelu,
                                                 bias=bias_sb[:, 5 * PC + kc:5 * PC + kc + 1])
                        else:
                            nc.scalar.activation(hc[:], fp[:], AF.Gelu)
                        wpc = w_sb2.tile([128, C], BF16, tag="wpc")
                        nc.sync.dma_start(wpc[:], wproj_d[kc * 128:(kc + 1) * 128, :])
                        hcs[kc] = (hc, wpc)
                        # delay proj by one kc so gelu overlaps the next FC
                        if kc >= 1:
                            hcp, wpcp = hcs.pop(kc - 1)
                            for oc in range(PC):
                                nc.tensor.matmul(prs[oc][:],
                                                 wpcp[:, oc * 128:(oc + 1) * 128],
                                                 hcp[:], start=(kc - 1 == 0), stop=False)
                    hcp, wpcp = hcs.pop(PC4 - 1)
                    for oc in range(PC):
                        nc.tensor.matmul(prs[oc][:], wpcp[:, oc * 128:(oc + 1) * 128],
                                         hcp[:], start=False, stop=True)
                    for oc in range(PC):
                        if has_proj_bias:
                            nc.scalar.activation(prs[oc][:], prs[oc][:], AF.Identity,
                                                 bias=bias_sb[:, 4 * PC + oc:4 * PC + oc + 1])
                        nc.vector.tensor_tensor(out_sb[:, oc, :], prs[oc][:], x2[:, oc, :],
                                                ALU.add)
            nc.sync.dma_start(out_d.rearrange("(o p) t -> p o t", p=128), out_sb[:])

    nc.compile()
    return nc


_CACHE = {}


def _get_program(flags, reps=1):
    key = (flags, reps)
    if key not in _CACHE:
        _CACHE[key] = _build(*flags, reps=reps)
    return _CACHE[key]


def kernel(**inputs) -> np.ndarray:
    x = np.asarray(inputs["x"], dtype=np.float32)
    padding_mask = np.asarray(inputs["padding_mask"])
    ln1_s = np.asarray(inputs["ln1_scale"], dtype=np.float32)
    ln1_b = np.asarray(inputs["ln1_bias"], dtype=np.float32)
    ln2_s = np.asarray(inputs["ln2_scale"], dtype=np.float32)
    ln2_b = np.asarray(inputs["ln2_bias"], dtype=np.float32)
    Wq = np.asarray(inputs["Wq"], dtype=np.float32)
    Wk = np.asarray(inputs["Wk"], dtype=np.float32)
    Wv = np.asarray(inputs["Wv"], dtype=np.float32)
    bq = np.asarray(inputs["bq"], dtype=np.float32)
    bk = np.asarray(inputs["bk"], dtype=np.float32)
    bv = np.asarray(inputs["bv"], dtype=np.float32)
    Wo = np.asarray(inputs["Wo"], dtype=np.float32)
    bo = np.asarray(inputs["bo"], dtype=np.float32)
    Wfc = np.asarray(inputs["Wfc"], dtype=np.float32)
    bfc = np.asarray(inputs["bfc"], dtype=np.float32)
    Wproj = np.asarray(inputs["Wproj"], dtype=np.float32)
    bproj = np.asarray(inputs["bproj"], dtype=np.float32)

    sc_q = 1.0 / np.sqrt(HD)
    Wq_f = Wq.transpose(1, 0, 2).reshape(C, C)
    Wk_f = Wk.transpose(1, 0, 2).reshape(C, C)
    Wv_f = Wv.transpose(1, 0, 2).reshape(C, C)
    wq_eff = (ln1_s[:, None] * Wq_f * sc_q).astype(ml_dtypes.bfloat16)
    wk_eff = (ln1_s[:, None] * Wk_f).astype(ml_dtypes.bfloat16)
    wv_eff = (ln1_s[:, None] * Wv_f).astype(ml_dtypes.bfloat16)
    bq_eff = (ln1_b @ Wq_f) * sc_q + bq.reshape(C) * sc_q
    bk_eff = ln1_b @ Wk_f + bk.reshape(C)
    bv_eff = ln1_b @ Wv_f + bv.reshape(C)
    wfc_eff = (ln2_s[:, None] * Wfc).astype(ml_dtypes.bfloat16)
    bfc_eff = ln2_b @ Wfc + bfc
    wfc_pre = np.ascontiguousarray(
        wfc_eff.reshape(PC, 128, PC4, 128).transpose(2, 1, 0, 3))
    wproj_b = Wproj.astype(ml_dtypes.bfloat16)
    wo_pre = np.ascontiguousarray(
        Wo.reshape(H, HD, C).transpose(1, 0, 2)).astype(ml_dtypes.bfloat16)

    biases = np.concatenate([bq_eff, bk_eff, bv_eff, bo, bproj, bfc_eff])
    bias_pre = np.ascontiguousarray(biases.reshape(NBIAS, 128).T).astype(np.float32)
    brows = np.concatenate([bq_eff, bk_eff, bv_eff]).astype(ml_dtypes.bfloat16)[None, :]

    has_qkv_bias = bool(np.abs(np.concatenate([bq_eff, bk_eff, bv_eff])).max() > 0)
    has_o_bias = bool(np.abs(bo).max() > 0)
    has_proj_bias = bool(np.abs(bproj).max() > 0)
    has_fc_bias = bool(np.abs(bfc_eff).max() > 0)
    has_mask = bool(padding_mask.any())

    nc = _get_program((has_qkv_bias, has_o_bias, has_proj_bias, has_fc_bias, has_mask))

    shared = {
        "wq": wq_eff, "wk": wk_eff, "wv": wv_eff, "wo": wo_pre,
        "wfc": wfc_pre, "wproj": wproj_b, "biases": bias_pre,
        "bias_rows": brows,
    }
    in_maps = []
    for c in range(NCORES):
        b, qo = c // (NCORES // B), (c % (NCORES // B)) * TQ
        xr = np.roll(x[b], -qo, axis=0)
        x_fm = np.ascontiguousarray(xr.T)
        x_bf = x_fm.astype(ml_dtypes.bfloat16)
        mrow = np.roll(padding_mask[b], -qo)
        maskb = np.ascontiguousarray(
            np.where(mrow, -1e30, 0.0).astype(np.float32).reshape(NSC, 128).T)
        in_maps.append({**shared, "x_fm": x_fm, "x_bf": x_bf, "maskb": maskb})

    res = run_bass_kernel_spmd(nc, in_maps, core_ids=list(range(NCORES)))

    out = np.empty((B, T, C), dtype=np.float32)
    for c in range(NCORES):
        b, qo = c // (NCORES // B), (c % (NCORES // B)) * TQ
        out[b, qo:qo + TQ, :] = res.results[c]["out_fm"].T
    return out


# revision 27
# speedup vs baseline: 1.3298x; 1.2953x over previous
"""Trainium2 Bass kernel for a GPT-style transformer block (B=2,T=2048,C=768,H=12).

Sharding: 8 cores; core c handles batch b=c//4, query block qo=(c%4)*512.
Each core gets its batch's x feature-major [C,T], rolled so its 512 query
tokens are columns 0:512.  K/V are computed for all 2048 keys (duplicated
across the 4 cores of a batch); Q/attention/MLP only for the 512 queries.

Perf structure (the PE only reaches 2.4GHz when its instruction stream is
gapless, else it runs at the 1.2GHz mid p-state):
 - LN1 stats are interleaved with the V projection stream.
 - Only K chunk oc=0 is computed up front; chunks oc=1..5 are spread through
   the attention inner loop (~1.5 matmuls/iter) as PE filler while the
   softmax exp runs on the Act engine.
 - Scores for iteration i+1 are emitted before AV of iteration i (software
   pipelining) so the PE never waits on the exp.
 - Denominators: augmented-V ones column -> row 64 of the AV psum; copied
   with the head's y to SBUF, broadcast via a rank-1 PE matmul, then a
   64-partition DVE reciprocal (single-partition reciprocal costs 3.3us).
 - MLP: bf16 weights, gelu delayed one kc so PE streams FC/proj gaplessly.
"""
import sys

sys.path.insert(0, "/opt/trn_rl_repo")

import numpy as np
import ml_dtypes

import concourse.bass as bass
import concourse.tile as tile
from concourse import bacc, mybir
from concourse.bass_utils import run_bass_kernel_spmd

F32 = mybir.dt.float32
F32R = mybir.dt.float32r
BF16 = mybir.dt.bfloat16
AF = mybir.ActivationFunctionType
ALU = mybir.AluOpType

B, T, C, H = 2, 2048, 768, 12
HD = C // H             # 64
C4 = 4 * C              # 3072
EPS = 1e-5
NCORES = 8
TQ = (B * T) // NCORES  # 512
PC = C // 128           # 6
PC4 = C4 // 128         # 24
NT4 = T // 512          # 4
NSC = T // 128          # 16
NBIAS = (5 * C + C4) // 128  # 54


def _build(has_qkv_bias, has_o_bias, has_proj_bias, has_fc_bias, has_mask, reps=1):
    has_bias_any = has_qkv_bias or has_o_bias or has_proj_bias or has_fc_bias
    nc = bacc.Bacc()

    x_d = nc.dram_tensor("x_fm", [C, T], F32, kind="ExternalInput")
    xb_d = nc.dram_tensor("x_bf", [C, T], BF16, kind="ExternalInput")
    wq_d = nc.dram_tensor("wq", [C, C], BF16, kind="ExternalInput")
    wk_d = nc.dram_tensor("wk", [C, C], BF16, kind="ExternalInput")
    wv_d = nc.dram_tensor("wv", [C, C], BF16, kind="ExternalInput")
    wo_d = nc.dram_tensor("wo", [HD + 1, H, C], BF16, kind="ExternalInput")
    wfc_d = nc.dram_tensor("wfc", [PC4, 128, PC, 128], BF16, kind="ExternalInput")
    wproj_d = nc.dram_tensor("wproj", [C4, C], BF16, kind="ExternalInput")
    bias_d = nc.dram_tensor("biases", [128, NBIAS], F32, kind="ExternalInput")
    brow_d = nc.dram_tensor("bias_rows", [1, 3 * C], BF16, kind="ExternalInput")
    mask_d = nc.dram_tensor("maskb", [128, NSC], F32, kind="ExternalInput")
    out_d = nc.dram_tensor("out_fm", [C, TQ], F32, kind="ExternalOutput")

    x_pot = x_d.rearrange("(o p) t -> p o t", p=128)
    xb_pot = xb_d.rearrange("(o p) t -> p o t", p=128)

    with tile.TileContext(nc) as tc:
      for _rep in range(reps):
        with tc.tile_pool(name=f"const{_rep}", bufs=1) as const, \
             tc.tile_pool(name=f"persist{_rep}", bufs=1) as persist:

            # ---------------- constants ----------------
            ones_f = const.tile([128, 1], F32)
            nc.vector.memset(ones_f[:], 1.0)
            ones_col_b = const.tile([128, 1], BF16)
            nc.vector.memset(ones_col_b[:], 1.0)
            ones_col_r = const.tile([128, 1], F32R)
            nc.vector.tensor_copy(ones_col_r[:], ones_f[:])
            onesr_f = const.tile([1, 128], F32)
            nc.vector.memset(onesr_f[:], 1.0)
            ones_row = const.tile([1, 128], F32R)
            nc.vector.tensor_copy(ones_row[:], onesr_f[:])
            ones_row_b = const.tile([1, 128], BF16)
            nc.vector.memset(ones_row_b[:], 1.0)

            if has_bias_any:
                bias_sb = const.tile([128, NBIAS], F32)
                nc.sync.dma_start(bias_sb[:], bias_d[:, :])
            if has_mask:
                mask_sb = const.tile([128, NSC], F32)
                nc.sync.dma_start(mask_sb[:], mask_d[:, :])
            if has_qkv_bias:
                brow_sb = const.tile([1, 3 * C], BF16)
                nc.sync.dma_start(brow_sb[:], brow_d[:, :])

            x_q = persist.tile([128, PC, TQ], F32)   # exact residual copy
            nc.sync.dma_start(x_q[:], x_pot[:, :, 0:TQ])

            with tc.tile_pool(name=f"attp{_rep}", bufs=1) as attp:
                q_bf = attp.tile([128, PC, TQ], BF16)
                k_bf = attp.tile([128, PC, T], BF16)
                vt_aug = attp.tile([128, NSC, H * (HD + 1)], BF16)
                x_bf = attp.tile([128, PC, T], BF16)
                nc.sync.dma_start(x_bf[:], xb_pot[:, :, :])
                istd_b = attp.tile([128, T], BF16)
                istd_col = attp.tile([128, NSC], F32)
                risd_r = attp.tile([1, T], BF16)  # sqrt(var+eps) (bias path only)
                wk_sb = attp.tile([128, PC, C], BF16)
                nc.sync.dma_start(wk_sb[:], wk_d.rearrange("(o p) m -> p o m", p=128))

                # ======== phases A+B: LN1 stats + V/Q + K(oc=0) ========
                with tc.tile_pool(name=f"rtmp{_rep}", bufs=3) as rtmp, \
                     tc.tile_pool(name=f"sqp{_rep}", bufs=4) as sqp, \
                     tc.tile_pool(name=f"wcyc{_rep}", bufs=2) as wcyc, \
                     tc.tile_pool(name=f"st_ps{_rep}", bufs=1, space="PSUM") as st_ps, \
                     tc.tile_pool(name=f"p12_ps{_rep}", bufs=2, space="PSUM") as p12_ps, \
                     tc.tile_pool(name=f"vq_ps{_rep}", bufs=2, space="PSUM") as vq_ps:

                    wv_sb = wcyc.tile([128, PC, C], BF16, tag="w")
                    nc.sync.dma_start(wv_sb[:], wv_d.rearrange("(o p) m -> p o m", p=128))

                    # --- column sums p1 for all 4 chunks (PE back-to-back) ---
                    p1s = []
                    for t4 in range(NT4):
                        sl = slice(t4 * 512, (t4 + 1) * 512)
                        p1 = p12_ps.tile([1, 512], F32, tag="p1")
                        for j in range(PC):
                            nc.tensor.matmul(p1[:], ones_col_b[:], x_bf[:, j, sl],
                                             start=(j == 0), stop=(j == PC - 1))
                        p1s.append(p1)
                    # --- -mean broadcast + centering (DVE/Pool) per chunk ---
                    nm_sbs = []
                    for t4 in range(NT4):
                        sl = slice(t4 * 512, (t4 + 1) * 512)
                        negmu_c = rtmp.tile([1, 512], BF16, tag="rtb")
                        nc.vector.tensor_scalar_mul(negmu_c[:], p1s[t4][:], -1.0 / C)
                        nm_ps = st_ps.tile([128, 512], F32, tag="nm")
                        nc.tensor.matmul(nm_ps[:], ones_row_b[:], negmu_c[:],
                                         start=True, stop=True)
                        nm_sb = sqp.tile([128, 512], BF16, tag="nmsb")
                        nc.vector.tensor_copy(nm_sb[:], nm_ps[:])
                        nm_sbs.append(nm_sb)
                        for j in range(PC):
                            eng = nc.vector if j < 2 else nc.gpsimd
                            eng.tensor_tensor(x_bf[:, j, sl], x_bf[:, j, sl],
                                              nm_sb[:], ALU.add)

                    # V projection emission helper: 32 half-blocks of 6 matmuls
                    def v_half(sc, half):
                        ssl = slice(sc * 128, (sc + 1) * 128)
                        if half == 0:
                            nc.gpsimd.memset(
                                vt_aug[:, sc, :].rearrange("p (h e) -> p h e", e=HD + 1)[:, :, 0:1],
                                1.0)
                        hsl = slice(half * 384, (half + 1) * 384)
                        vp = vq_ps.tile([128, 512], F32, tag="pp", name="vp")[:, 0:384]
                        for j in range(PC):
                            nc.tensor.matmul(vp[:], x_bf[:, j, ssl], wv_sb[:, j, hsl],
                                             start=(j == 0),
                                             stop=(j == PC - 1 and not has_qkv_bias))
                        if has_qkv_bias:
                            nc.tensor.matmul(
                                vp[:], risd_r[:, ssl],
                                brow_sb[:, 2 * C + half * 384:2 * C + (half + 1) * 384],
                                start=False, stop=True)
                        dst = vt_aug[:, sc, :].rearrange("p (h e) -> p h e", e=HD + 1)[
                            :, half * 6:(half + 1) * 6, 1:HD + 1]
                        if half == 0:
                            nc.scalar.activation(
                                dst, vp[:].rearrange("p (h e) -> p h e", e=HD),
                                AF.Copy, scale=istd_col[:, sc:sc + 1])
                        else:
                            nc.vector.tensor_scalar(
                                dst, vp[:].rearrange("p (h e) -> p h e", e=HD),
                                istd_col[:, sc:sc + 1], None, ALU.mult)

                    vq = [(sc, half) for sc in range(NSC) for half in range(2)]
                    vpos = 0

                    def emit_v(n):
                        nonlocal vpos
                        for _ in range(n):
                            if vpos < len(vq):
                                v_half(*vq[vpos])
                                vpos += 1

                    # --- variance / istd per chunk, V interleaved as filler ---
                    emit_v(3)
                    for t4 in range(NT4):
                        sl = slice(t4 * 512, (t4 + 1) * 512)
                        p2 = p12_ps.tile([1, 512], F32, tag="p2")
                        for j in range(PC):
                            xsq = sqp.tile([128, 512], BF16, tag="xsq")
                            eng = (nc.scalar if j < 3 else
                                   (nc.vector if j < 5 else nc.gpsimd))
                            if eng is nc.scalar:
                                nc.scalar.activation(xsq[:], x_bf[:, j, sl], AF.Square)
                            else:
                                eng.tensor_tensor(xsq[:], x_bf[:, j, sl], x_bf[:, j, sl],
                                                  ALU.mult)
                            nc.tensor.matmul(p2[:], ones_col_b[:], xsq[:],
                                             start=(j == 0), stop=(j == PC - 1))
                        var_c = rtmp.tile([1, 512], F32, tag="rt")
                        nc.vector.tensor_scalar(var_c[:], p2[:], 1.0 / C, EPS,
                                                ALU.mult, ALU.add)
                        lnv_c = rtmp.tile([1, 512], F32, tag="rt")
                        nc.scalar.activation(lnv_c[:], var_c[:], AF.Ln)
                        istd_c = rtmp.tile([1, 512], F32, tag="rt")
                        nc.scalar.activation(istd_c[:], lnv_c[:], AF.Exp, scale=-0.5)
                        istd_cb = rtmp.tile([1, 512], BF16, tag="rtb")
                        nc.vector.tensor_copy(istd_cb[:], istd_c[:])
                        if has_qkv_bias:
                            nc.scalar.activation(risd_r[:, sl], lnv_c[:], AF.Exp, scale=0.5)
                        emit_v(2)
                        # broadcast istd across partitions
                        bp = st_ps.tile([128, 512], F32, tag="bp")
                        nc.tensor.matmul(bp[:], ones_row_b[:], istd_cb[:],
                                         start=True, stop=True)
                        nc.scalar.activation(istd_b[:, sl], bp[:], AF.Copy)
                        # istd token-major [128, 4 cols] for the V copyback
                        for o in range(4):
                            nc.sync.dma_start(istd_col[:, t4 * 4 + o:t4 * 4 + o + 1],
                                              istd_c[0:1, o * 128:(o + 1) * 128])
                        emit_v(2)
                    emit_v(len(vq))

                    # ---- Q (queries only) ----
                    wq_sb = wcyc.tile([128, PC, C], BF16, tag="w")
                    nc.sync.dma_start(wq_sb[:], wq_d.rearrange("(o p) m -> p o m", p=128))
                    for oc in range(PC):
                        qp = vq_ps.tile([128, 512], F32, tag="pp", name="qp")
                        for j in range(PC):
                            nc.tensor.matmul(qp[:], wq_sb[:, j, oc * 128:(oc + 1) * 128],
                                             x_bf[:, j, 0:TQ], start=(j == 0),
                                             stop=(j == PC - 1 and not has_qkv_bias))
                        if has_qkv_bias:
                            nc.tensor.matmul(qp[:], brow_sb[:, oc * 128:(oc + 1) * 128],
                                             risd_r[:, 0:TQ], start=False, stop=True)
                        nc.vector.tensor_tensor(q_bf[:, oc, :], qp[:], istd_b[:, 0:TQ],
                                                ALU.mult)
                    # ---- K chunk oc=0 only (rest run inside attention) ----
                    for t4 in range(NT4):
                        sl = slice(t4 * 512, (t4 + 1) * 512)
                        kp = vq_ps.tile([128, 512], F32, tag="pp", name="kp")
                        for j in range(PC):
                            nc.tensor.matmul(kp[:], wk_sb[:, j, 0:128],
                                             x_bf[:, j, sl], start=(j == 0),
                                             stop=(j == PC - 1 and not has_qkv_bias))
                        if has_qkv_bias:
                            nc.tensor.matmul(kp[:], brow_sb[:, C:C + 128],
                                             risd_r[:, sl], start=False, stop=True)
                        nc.vector.tensor_tensor(k_bf[:, 0, sl], kp[:], istd_b[:, sl],
                                                ALU.mult)

                # ============ phase C: attention (K oc=1..5 interleaved) ====
                wo_sb = persist.tile([HD + 1, H, C], BF16)
                nc.sync.dma_start(wo_sb[:], wo_d[:, :, :])

                with tc.tile_pool(name=f"ypool{_rep}", bufs=1) as ypool:
                    y_sb = ypool.tile([HD + 1, H, TQ], BF16)
                    y_nm = ypool.tile([HD + 1, H, TQ], BF16)

                    with tc.tile_pool(name=f"sc_ps{_rep}", bufs=2, space="PSUM") as sc_ps, \
                         tc.tile_pool(name=f"y_psp{_rep}", bufs=2, space="PSUM") as y_psp, \
                         tc.tile_pool(name=f"rp_ps{_rep}", bufs=1, space="PSUM") as rp_ps, \
                         tc.tile_pool(name=f"kp_ps{_rep}", bufs=1, space="PSUM") as kp_ps, \
                         tc.tile_pool(name=f"attb{_rep}", bufs=3) as attb, \
                         tc.tile_pool(name=f"recb{_rep}", bufs=2) as recb:

                        def k_chunk_gen(oc):
                            # yields after each PE matmul; copybacks on DVE
                            for t4 in range(NT4):
                                sl = slice(t4 * 512, (t4 + 1) * 512)
                                kp = kp_ps.tile([128, 512], F32, tag="kp")
                                for j in range(PC):
                                    nc.tensor.matmul(
                                        kp[:], wk_sb[:, j, oc * 128:(oc + 1) * 128],
                                        x_bf[:, j, sl], start=(j == 0),
                                        stop=(j == PC - 1 and not has_qkv_bias))
                                    if j < PC - 1:
                                        yield
                                if has_qkv_bias:
                                    nc.tensor.matmul(
                                        kp[:], brow_sb[:, C + oc * 128:C + (oc + 1) * 128],
                                        risd_r[:, sl], start=False, stop=True)
                                nc.vector.tensor_tensor(k_bf[:, oc, sl], kp[:],
                                                        istd_b[:, sl], ALU.mult)
                                yield

                        def make_tail(h, yp):
                            def tail():
                                nc.vector.tensor_copy(y_sb[:, h, :], yp[:])
                                rp = rp_ps.tile([HD + 1, TQ], F32, tag="rp")
                                nc.tensor.matmul(rp[:], ones_row_b[:, 0:HD + 1],
                                                 y_sb[0:1, h, :],
                                                 start=True, stop=True)
                                rec = recb.tile([HD + 1, TQ], F32, tag="rec")
                                nc.vector.reciprocal(rec[:], rp[:])
                                nc.gpsimd.tensor_tensor(y_nm[:, h, :], y_sb[:, h, :],
                                                        rec[:], ALU.mult)
                            return tail

                        kgen = None
                        pending_tail = None
                        for h in range(H):
                            base = 64 * (h & 1)
                            ch = h // 2
                            if h < 10 and h % 2 == 0:
                                kgen = k_chunk_gen(1 + h // 2)
                            yp = y_psp.tile([HD + 1, TQ], F32, tag="yp")
                            prev_av = None
                            for scp in range(NSC // 2):
                                sp = sc_ps.tile([128, 2, 512], F32, tag="sp")
                                for i in range(2):
                                    sc = 2 * scp + i
                                    nc.tensor.matmul(
                                        sp[:, i, :],
                                        k_bf[base:base + HD, ch, sc * 128:(sc + 1) * 128],
                                        q_bf[base:base + HD, ch, :],
                                        start=True, stop=True)
                                att = attb.tile([128, 2, 512], BF16, tag="att")
                                if has_mask:
                                    for i in range(2):
                                        sc = 2 * scp + i
                                        nc.scalar.activation(att[:, i, :], sp[:, i, :], AF.Exp,
                                                             bias=mask_sb[:, sc:sc + 1])
                                else:
                                    nc.scalar.activation(att[:], sp[:], AF.Exp)
                                if prev_av is not None:
                                    prev_av()
                                if pending_tail is not None:
                                    pending_tail()
                                    pending_tail = None
                                if kgen is not None:
                                    for _ in range(2 if scp % 2 == 0 else 1):
                                        if next(kgen, "end") == "end":
                                            kgen = None
                                            break

                                def av(att=att, scp=scp):
                                    for i in range(2):
                                        sc = 2 * scp + i
                                        nc.tensor.matmul(yp[:], vt_aug[:, sc, 65 * h:65 * h + 65],
                                                         att[:, i, :],
                                                         start=(sc == 0), stop=(sc == NSC - 1))
                                prev_av = av
                            prev_av()
                            pending_tail = make_tail(h, yp)
                        pending_tail()

                    # ---- Wo + residual + LN2 stats interleave ----
                    x2 = persist.tile([128, PC, TQ], F32R)
                    with tc.tile_pool(name=f"dtmp{_rep}", bufs=2) as dtmp, \
                         tc.tile_pool(name=f"drow{_rep}", bufs=1) as drow:
                      with tc.tile_pool(name=f"wo_ps{_rep}", bufs=1, space="PSUM") as wo_ps, \
                           tc.tile_pool(name=f"xsqp{_rep}", bufs=3) as xsqp, \
                           tc.tile_pool(name=f"d_ps{_rep}", bufs=1, space="PSUM") as d_ps:
                        p1 = d_ps.tile([1, TQ], F32, tag="p1")
                        p2 = d_ps.tile([1, TQ], F32, tag="p2")
                        for oc in range(PC):
                            op = wo_ps.tile([128, TQ], F32, tag=f"op{oc}", name=f"op{oc}")
                            for hh in range(H):
                                nc.tensor.matmul(op[:], wo_sb[:, hh, oc * 128:(oc + 1) * 128],
                                                 y_nm[:, hh, :], start=(hh == 0),
                                                 stop=(hh == H - 1))
                            if has_o_bias:
                                nc.scalar.activation(op[:], op[:], AF.Identity,
                                                     bias=bias_sb[:, 3 * PC + oc:3 * PC + oc + 1])
                            nc.vector.tensor_tensor(x2[:, oc, :], op[:], x_q[:, oc, :],
                                                    ALU.add)
                            xsqa = xsqp.tile([128, TQ], F32R, tag="xsqa")
                            nc.scalar.activation(xsqa[:], x2[:, oc, :], AF.Square)
                            nc.tensor.matmul(p1[:], ones_col_r[:], x2[:, oc, :],
                                             start=(oc == 0), stop=(oc == PC - 1))
                            nc.tensor.matmul(p2[:], ones_col_r[:], xsqa[:],
                                             start=(oc == 0), stop=(oc == PC - 1))

                        # LN2 scalars that read p1/p2 (before d_ps closes)
                        mean2 = dtmp.tile([1, TQ], F32, tag="dt")
                        nc.vector.tensor_scalar_mul(mean2[:], p1[:], 1.0 / C)
                        msq2 = dtmp.tile([1, TQ], F32, tag="dt")
                        nc.vector.tensor_tensor(msq2[:], mean2[:], mean2[:], ALU.mult)
                        var2 = dtmp.tile([1, TQ], F32, tag="dt")
                        nc.vector.tensor_scalar(var2[:], p2[:], 1.0 / C, EPS,
                                                ALU.mult, ALU.add)
                        nc.vector.tensor_sub(var2[:], var2[:], msq2[:])
                        negmu2_r = drow.tile([1, TQ], F32R)
                        nc.vector.tensor_scalar_mul(negmu2_r[:], mean2[:], -1.0)

                      lnv2 = dtmp.tile([1, TQ], F32, tag="dt")
                      nc.scalar.activation(lnv2[:], var2[:], AF.Ln)
                      istd2 = dtmp.tile([1, TQ], F32, tag="dt")
                      nc.scalar.activation(istd2[:], lnv2[:], AF.Exp, scale=-0.5)
                      istd2_r = drow.tile([1, TQ], F32R)
                      nc.vector.tensor_copy(istd2_r[:], istd2[:])
                      xc_bf = persist.tile([128, PC, TQ], BF16)
                      with tc.tile_pool(name=f"d2_ps{_rep}", bufs=1, space="PSUM") as d2_ps:
                        bp2 = d2_ps.tile([128, TQ], F32, tag="bp2")
                        nc.tensor.matmul(bp2[:], ones_row[:], istd2_r[:],
                                         start=True, stop=True)
                        istd2_b = drow.tile([128, TQ], F32)
                        nc.scalar.activation(istd2_b[:], bp2[:], AF.Copy)
                        nm2_ps = d2_ps.tile([128, TQ], F32, tag="nm2")
                        nc.tensor.matmul(nm2_ps[:], ones_row[:], negmu2_r[:],
                                         start=True, stop=True)
                        nm2_b = drow.tile([128, TQ], F32)
                        nc.vector.tensor_copy(nm2_b[:], nm2_ps[:])
                        # center + scale + cast: xc = (x2 - mu) * istd  (bf16)
                        for j in range(PC):
                            xct = dtmp.tile([128, TQ], F32, tag="xct")
                            eng, eng2 = ((nc.vector, nc.gpsimd) if j % 2 == 0
                                         else (nc.gpsimd, nc.vector))
                            eng.tensor_tensor(xct[:], x2[:, j, :], nm2_b[:], ALU.add)
                            eng2.tensor_tensor(xc_bf[:, j, :], xct[:], istd2_b[:],
                                               ALU.mult)

            # ============ phase E: MLP ============
            out_sb = persist.tile([128, PC, TQ], F32)
            with tc.tile_pool(name=f"pr_ps{_rep}", bufs=1, space="PSUM") as pr_ps, \
                 tc.tile_pool(name=f"fc_ps{_rep}", bufs=2, space="PSUM") as fc_ps, \
                 tc.tile_pool(name=f"h_sb{_rep}", bufs=3) as h_sb, \
                 tc.tile_pool(name=f"w_sb2{_rep}", bufs=4) as w_sb2:
                    prs = [pr_ps.tile([128, TQ], F32, tag=f"pr{i}", name=f"pr{i}")
                           for i in range(PC)]
                    hcs = {}
                    for kc in range(PC4):
                        wfcc = w_sb2.tile([128, PC, 128], BF16, tag="wfcc")
                        nc.sync.dma_start(wfcc[:], wfc_d[kc])
                        fp = fc_ps.tile([128, TQ], F32, tag="fp")
                        for j in range(PC):
                            nc.tensor.matmul(fp[:], wfcc[:, j, :], xc_bf[:, j, :],
                                             start=(j == 0), stop=(j == PC - 1))
                        hc = h_sb.tile([128, TQ], BF16, tag="hc")
                        if has_fc_bias:
                            nc.scalar.activation(hc[:], fp[:], AF.Gelu,
                                                 bias=bias_sb[:, 5 * PC + kc:5 * PC + kc + 1])
                        else:
                            nc.scalar.activation(hc[:], fp[:], AF.Gelu)
                        wpc = w_sb2.tile([128, C], BF16, tag="wpc")
                        nc.sync.dma_start(wpc[:], wproj_d[kc * 128:(kc + 1) * 128, :])
                        hcs[kc] = (hc, wpc)
                        # delay proj by one kc so gelu overlaps the next FC
                        if kc >= 1:
                            hcp, wpcp = hcs.pop(kc - 1)
                            for oc in range(PC):
                                nc.tensor.matmul(prs[oc][:],
                                                 wpcp[:, oc * 128:(oc + 1) * 128],
                                                 hcp[:], start=(kc - 1 == 0), stop=False)
                    hcp, wpcp = hcs.pop(PC4 - 1)
                    for oc in range(PC):
                        nc.tensor.matmul(prs[oc][:], wpcp[:, oc * 128:(oc + 1) * 128],
                                         hcp[:], start=False, stop=True)
                    for oc in range(PC):
                        if has_proj_bias:
                            nc.scalar.activation(prs[oc][:], prs[oc][:], AF.Identity,
                                                 bias=bias_sb[:, 4 * PC + oc:4 * PC + oc + 1])
                        nc.vector.tensor_tensor(out_sb[:, oc, :], prs[oc][:], x2[:, oc, :],
                                                ALU.add)
            nc.sync.dma_start(out_d.rearrange("(o p) t -> p o t", p=128), out_sb[:])

    nc.compile()
    return nc


_CACHE = {}


def _get_program(flags, reps=1):
    key = (flags, reps)
    if key not in _CACHE:
        _CACHE[key] = _build(*flags, reps=reps)
    return _CACHE[key]


def kernel(**inputs) -> np.ndarray:
    x = np.asarray(inputs["x"], dtype=np.float32)
    padding_mask = np.asarray(inputs["padding_mask"])
    ln1_s = np.asarray(inputs["ln1_scale"], dtype=np.float32)
    ln1_b = np.asarray(inputs["ln1_bias"], dtype=np.float32)
    ln2_s = np.asarray(inputs["ln2_scale"], dtype=np.float32)
    ln2_b = np.asarray(inputs["ln2_bias"], dtype=np.float32)
    Wq = np.asarray(inputs["Wq"], dtype=np.float32)
    Wk = np.asarray(inputs["Wk"], dtype=np.float32)
    Wv = np.asarray(inputs["Wv"], dtype=np.float32)
    bq = np.asarray(inputs["bq"], dtype=np.float32)
    bk = np.asarray(inputs["bk"], dtype=np.float32)
    bv = np.asarray(inputs["bv"], dtype=np.float32)
    Wo = np.asarray(inputs["Wo"], dtype=np.float32)
    bo = np.asarray(inputs["bo"], dtype=np.float32)
    Wfc = np.asarray(inputs["Wfc"], dtype=np.float32)
    bfc = np.asarray(inputs["bfc"], dtype=np.float32)
    Wproj = np.asarray(inputs["Wproj"], dtype=np.float32)
    bproj = np.asarray(inputs["bproj"], dtype=np.float32)

    sc_q = 1.0 / np.sqrt(HD)
    Wq_f = Wq.transpose(1, 0, 2).reshape(C, C)
    Wk_f = Wk.transpose(1, 0, 2).reshape(C, C)
    Wv_f = Wv.transpose(1, 0, 2).reshape(C, C)
    wq_eff = (ln1_s[:, None] * Wq_f * sc_q).astype(ml_dtypes.bfloat16)
    wk_eff = (ln1_s[:, None] * Wk_f).astype(ml_dtypes.bfloat16)
    wv_eff = (ln1_s[:, None] * Wv_f).astype(ml_dtypes.bfloat16)
    bq_eff = (ln1_b @ Wq_f) * sc_q + bq.reshape(C) * sc_q
    bk_eff = ln1_b @ Wk_f + bk.reshape(C)
    bv_eff = ln1_b @ Wv_f + bv.reshape(C)
    wfc_eff = (ln2_s[:, None] * Wfc).astype(ml_dtypes.bfloat16)
    bfc_eff = ln2_b @ Wfc + bfc
    wfc_pre = np.ascontiguousarray(
        wfc_eff.reshape(PC, 128, PC4, 128).transpose(2, 1, 0, 3))
    wproj_b = Wproj.astype(ml_dtypes.bfloat16)
    wo_hd = Wo.reshape(H, HD, C).transpose(1, 0, 2)
    wo_pre = np.ascontiguousarray(
        np.concatenate([np.zeros((1, H, C), np.float32), wo_hd],
                       axis=0)).astype(ml_dtypes.bfloat16)

    biases = np.concatenate([bq_eff, bk_eff, bv_eff, bo, bproj, bfc_eff])
    bias_pre = np.ascontiguousarray(biases.reshape(NBIAS, 128).T).astype(np.float32)
    brows = np.concatenate([bq_eff, bk_eff, bv_eff]).astype(ml_dtypes.bfloat16)[None, :]

    has_qkv_bias = bool(np.abs(np.concatenate([bq_eff, bk_eff, bv_eff])).max() > 0)
    has_o_bias = bool(np.abs(bo).max() > 0)
    has_proj_bias = bool(np.abs(bproj).max() > 0)
    has_fc_bias = bool(np.abs(bfc_eff).max() > 0)
    has_mask = bool(padding_mask.any())

    nc = _get_program((has_qkv_bias, has_o_bias, has_proj_bias, has_fc_bias, has_mask))

    shared = {
        "wq": wq_eff, "wk": wk_eff, "wv": wv_eff, "wo": wo_pre,
        "wfc": wfc_pre, "wproj": wproj_b, "biases": bias_pre,
        "bias_rows": brows,
    }
    in_maps = []
    for c in range(NCORES):
        b, qo = c // (NCORES // B), (c % (NCORES // B)) * TQ
        xr = np.roll(x[b], -qo, axis=0)
        x_fm = np.ascontiguousarray(xr.T)
        x_bf = x_fm.astype(ml_dtypes.bfloat16)
        mrow = np.roll(padding_mask[b], -qo)
        maskb = np.ascontiguousarray(
            np.where(mrow, -1e30, 0.0).astype(np.float32).reshape(NSC, 128).T)
        in_maps.append({**shared, "x_fm": x_fm, "x_bf": x_bf, "maskb": maskb})

    res = run_bass_kernel_spmd(nc, in_maps, core_ids=list(range(NCORES)))

    out = np.empty((B, T, C), dtype=np.float32)
    for c in range(NCORES):
        b, qo = c // (NCORES // B), (c % (NCORES // B)) * TQ
        out[b, qo:qo + TQ, :] = res.results[c]["out_fm"].T
    return out
